# revision 25
# baseline (speedup 1.0000x reference)
"""Trainium2 Bass kernel for nn_Discriminator (NeuralSort + MLP discriminator).

Computes, for x [64, 1024]:
    P_hat = softmax_j((scaling[i]*x_j - Bsum_j) / TAU)   (per sample)
    xs    = P_hat @ x
    out   = leaky(leaky(xs@W1.T + b1)@W2.T + b2) @ W3.T + b3

Data parallel over 8 NeuronCores: 8 samples per core.

All PE matmul streams run in bf16 with split-precision operands (bf16
products accumulate exactly in fp32 PSUM; fp32 matmuls are 4x slower on
TRN2):
  - logits (argexp): K=9  (t_h,t_m,t_l,t_h,t_m,B_h,B_m,B_l,1) x
                          (a_h,a_h,a_h,a_l,a_l,-1,-1,-1,m)
    t 3-way split, Bsum 3-way split, a = a_h+a_l exact (integers),
    m single bf16 (errors in m cancel exactly in the softmax ratio).
  - row max:  K=4 (a;a;-1;-1) x (t_h;t_m;B_h;B_m), exact only on every
    4th row; the group max is a valid softmax shift (slack < 50 << 88).
  - num/den: lhsT = (s_h, s_l, 1) columns, rhs = E (bf16 exp output);
    num = num_h + num_l recombined after the column flatten.
  - MLP: W = W_h + W_l bf16 splits (host), activations split on device;
    dropped l*l term ~1e-5 relative.
G/Bsum (the only fp32-sensitive reduction) runs on ACT (fused
Abs-activation + accum) and DVE (subtract + abs-add-reduce), not PE.
"""

import os

import numpy as np

import concourse.bass as bass
import concourse.bacc as bacc
import concourse.tile as tile
from concourse import mybir
from concourse.bass_utils import run_bass_kernel_spmd

F32 = mybir.dt.float32
BF16 = mybir.dt.bfloat16
ALU = mybir.AluOpType
ACTF = mybir.ActivationFunctionType

B, D = 64, 1024
NCORES = 8
S = B // NCORES          # samples per core
T = D // 128             # j tiles per sample
TAU = 1.0
NEG_SLOPE = 0.01
A_ACT = int(os.environ.get("A_ACT", "4"))  # G+Bsum tiles handled by ACT (rest DVE)
GPS_SUB = os.environ.get("GPS_SUB", "0") == "1"  # diff on gpsimd for DVE tiles
SKIP = set(os.environ.get("SKIP", "").split(","))  # timing ablations
MAXSTRIDE = 4            # compute exact row max every MAXSTRIDE rows
QT = D // (128 * MAXSTRIDE)  # packed max tiles per sample (2)


def bf_split(x, n):
    """Split x into n bf16 parts (sum of parts -> x with ~8n mantissa bits)."""
    import ml_dtypes
    parts = []
    r = np.asarray(x, np.float32)
    for _ in range(n):
        p = r.astype(ml_dtypes.bfloat16)
        parts.append(p)
        r = r - p.astype(np.float32)
    return parts


def build_nc(loop_n: int = 1):
    nc = bacc.Bacc("TRN2", target_bir_lowering=False, debug=False,
                   enable_asserts=False, num_devices=NCORES)

    xs8 = nc.dram_tensor("xs8", [S, D], F32, kind="ExternalInput")
    l9i = nc.dram_tensor("l9i", [S, 9, D], BF16, kind="ExternalInput")
    l4i = nc.dram_tensor("l4i", [S, 4, D], BF16, kind="ExternalInput")
    swg_i = nc.dram_tensor("swg", [S, 128, T], F32, kind="ExternalInput")
    sw3_i = nc.dram_tensor("sw3", [S, 128, 3 * T], BF16, kind="ExternalInput")
    r9c_i = nc.dram_tensor("r9c", [9, D], BF16, kind="ExternalInput")
    a4d_i = nc.dram_tensor("a4d", [4, D // MAXSTRIDE], BF16, kind="ExternalInput")
    onesb_i = nc.dram_tensor("onesb", [1, S], BF16, kind="ExternalInput")
    ones_i = nc.dram_tensor("ones8", [1, S], F32, kind="ExternalInput")
    w1h_i = nc.dram_tensor("w1h", [D, D], BF16, kind="ExternalInput")
    w1l_i = nc.dram_tensor("w1l", [D, D], BF16, kind="ExternalInput")
    w2h_i = nc.dram_tensor("w2h", [D, D], BF16, kind="ExternalInput")
    w2l_i = nc.dram_tensor("w2l", [D, D], BF16, kind="ExternalInput")
    w3t_i = nc.dram_tensor("w3t", [D, 2], F32, kind="ExternalInput")
    b1_i = nc.dram_tensor("b1r", [1, D], BF16, kind="ExternalInput")
    b2_i = nc.dram_tensor("b2r", [1, D], BF16, kind="ExternalInput")
    b3_i = nc.dram_tensor("b3r", [1, 2], F32, kind="ExternalInput")
    id8_i = nc.dram_tensor("id8", [S, S], F32, kind="ExternalInput")
    out_t = nc.dram_tensor("out", [S, 2], F32, kind="ExternalOutput")

    args = (xs8, l9i, l4i, swg_i, sw3_i, r9c_i, a4d_i, onesb_i, ones_i,
            w1h_i, w1l_i, w2h_i, w2l_i, w3t_i, b1_i, b2_i, b3_i, id8_i, out_t)
    with tile.TileContext(nc) as tc:
        _body(nc, tc, args, loop_n)
    nc.finalize()
    return nc


def _body(nc, tc, args, loop_n):
    (xs8, l9i, l4i, swg_i, sw3_i, r9c_i, a4d_i, onesb_i, ones_i,
     w1h_i, w1l_i, w2h_i, w2l_i, w3t_i, b1_i, b2_i, b3_i, id8_i, out_t) = args
    from contextlib import ExitStack
    ctx = ExitStack()
    with ctx:
        consts = ctx.enter_context(tc.tile_pool(name="consts", bufs=1))
        per_s = ctx.enter_context(tc.tile_pool(name="per_s", bufs=4))
        big = ctx.enter_context(tc.tile_pool(name="big", bufs=3))
        epool = ctx.enter_context(tc.tile_pool(name="epool", bufs=4))
        dram = ctx.enter_context(tc.tile_pool(name="dram", bufs=4, space="DRAM"))

        # ---- constants resident in SBUF ----
        r9c = consts.tile([9, D], BF16)
        nc.sync.dma_start(out=r9c, in_=r9c_i[:, :])
        a4d = consts.tile([4, D // MAXSTRIDE], BF16)
        nc.sync.dma_start(out=a4d, in_=a4d_i[:, :])
        onesb = consts.tile([1, S], BF16)
        nc.sync.dma_start(out=onesb, in_=onesb_i[:, :])
        ones8 = consts.tile([1, S], F32)
        nc.sync.dma_start(out=ones8, in_=ones_i[:, :])
        wtiles = {}
        for nm, hnd in (("w1h", w1h_i), ("w1l", w1l_i),
                        ("w2h", w2h_i), ("w2l", w2l_i)):
            wt = consts.tile([128, T * D], BF16, tag=nm)
            for g in range(T):
                nc.scalar.dma_start(out=wt[:, g * D:(g + 1) * D],
                                    in_=hnd[128 * g:128 * (g + 1), :])
            wtiles[nm] = wt
        w3sb = consts.tile([128, 2 * T], F32)
        for g in range(T):
            nc.scalar.dma_start(out=w3sb[:, 2 * g:2 * g + 2],
                                in_=w3t_i[128 * g:128 * (g + 1), :])
        b1r = consts.tile([1, D], BF16, tag="b1r")
        nc.sync.dma_start(out=b1r, in_=b1_i[:, :])
        b2r = consts.tile([1, D], BF16, tag="b2r")
        nc.sync.dma_start(out=b2r, in_=b2_i[:, :])
        b3r = consts.tile([1, 2], F32)
        nc.sync.dma_start(out=b3r, in_=b3_i[:, :])
        id8 = consts.tile([S, S], F32)
        nc.sync.dma_start(out=id8, in_=id8_i[:, :])

        # persistent per-core accumulators (columns, col index g*S + b)
        nhT = consts.tile([128, S * T], F32, tag="nhT")
        nlT = consts.tile([128, S * T], F32, tag="nlT")
        denT = consts.tile([128, S * T], F32, tag="denT")

        def one_rep():
            with (
                tc.tile_pool(name="pbig", bufs=3, space="PSUM") as pbig,
                tc.tile_pool(name="pnd", bufs=1, space="PSUM") as pnd,
            ):
                for b in range(S):
                    fr = _sample_front(nc, tc, b, xs8, l9i, l4i, swg_i,
                                       sw3_i, r9c, a4d, per_s, big, epool,
                                       dram, pbig)
                    _sample_back(nc, tc, b, fr, per_s, epool, dram,
                                 pbig, pnd, nhT, nlT, denT)
            if "mlp" not in SKIP:
                with tc.tile_pool(name="pmlp", bufs=2, space="PSUM") as pmlp:
                    _mlp(nc, tc, per_s, big, dram, pmlp, nhT, nlT, denT,
                         wtiles, w3sb, b1r, b2r, b3r, onesb, ones8, id8, out_t)
            else:
                osb = big.tile([S, 2], F32, tag="osb")
                nc.vector.tensor_copy(out=osb, in_=nhT[0:S, 0:2])
                nc.sync.dma_start(out=out_t[:, :], in_=osb)

        if loop_n == 1:
            one_rep()
        else:
            with tc.For_i(0, loop_n, 1):
                one_rep()


def _flatten(nc, dram, cols, dst, k, dt):
    """cols [128, k] -> dst row-ish AP [*, 128*k] with flat[128*g+p]=cols[p,g]."""
    scr = dram.tile([128, k], dt, tag=f"scr{k}_{dt}")
    nc.sync.dma_start(out=scr, in_=cols)
    sap = scr[:, :]
    nc.sync.dma_start(out=dst, in_=bass.AP(
        tensor=sap.tensor, offset=sap.offset, ap=[[1, k], [k, 128]]))


def _sample_front(nc, tc, b, xs8, l9i, l4i, swg_i, sw3_i, r9c, a4d,
                  per_s, big, epool, dram, pbig):
    # ---- per-sample loads ----
    l9 = per_s.tile([9, D], BF16, tag="l9")
    nc.sync.dma_start(out=l9, in_=l9i[b, :, :])
    l4 = per_s.tile([4, D], BF16, tag="l4")
    nc.sync.dma_start(out=l4, in_=l4i[b, :, :])
    swg = per_s.tile([128, T], F32, tag="swg")
    nc.sync.dma_start(out=swg, in_=swg_i[b, :, :])
    sw3 = per_s.tile([128, 3 * T], BF16, tag="sw3")
    nc.sync.dma_start(out=sw3, in_=sw3_i[b, :, :])

    # S_bcast: x[b]/TAU broadcast to 128 partitions
    sbc = big.tile([128, D], F32, tag="sbc")
    src = xs8[b:b + 1, :]
    nc.sync.dma_start(out=sbc, in_=bass.AP(
        tensor=src.tensor, offset=src.offset, ap=[[0, 128]] + src.ap[1:]))

    # ---- G + Bsum (fp32) ----
    if "g" not in SKIP:
        bcols = per_s.tile([128, T], F32, tag="bcols")
        for g in range(T):
            if g < A_ACT:
                gs = big.tile([128, D], F32, tag="gscr")
                nc.scalar.activation(out=gs, in_=sbc, func=ACTF.Abs,
                                     bias=swg[:, g:g + 1], scale=-1.0,
                                     accum_out=bcols[:, g:g + 1])
            else:
                ds = big.tile([128, D], F32, tag="gscr")
                if GPS_SUB:
                    nc.gpsimd.tensor_scalar_sub(out=ds, in0=sbc,
                                                scalar1=swg[:, g:g + 1])
                else:
                    nc.vector.tensor_scalar_sub(out=ds, in0=sbc,
                                                scalar1=swg[:, g:g + 1])
                nc.vector.tensor_reduce(out=bcols[:, g:g + 1], in_=ds,
                                        axis=mybir.AxisListType.X, op=ALU.add,
                                        apply_absolute_value=True)

        # split Bsum cols into 3 bf16 parts
        bh = per_s.tile([128, T], BF16, tag="bh")
        nc.vector.tensor_copy(out=bh, in_=bcols)
        bmf = per_s.tile([128, T], F32, tag="bmf")
        nc.vector.tensor_sub(out=bmf, in0=bcols, in1=bh)
        bm = per_s.tile([128, T], BF16, tag="bm")
        nc.vector.tensor_copy(out=bm, in_=bmf)
        blf = per_s.tile([128, T], F32, tag="blf")
        nc.vector.tensor_sub(out=blf, in0=bmf, in1=bm)
        bl = per_s.tile([128, T], BF16, tag="bl")
        nc.vector.tensor_copy(out=bl, in_=blf)

        # flatten splits into l9 rows 5-7 and l4 rows 2-3
        _flatten(nc, dram, bh, l9[5:6, :], T, BF16)
        _flatten(nc, dram, bm, l9[6:7, :], T, BF16)
        _flatten(nc, dram, bl, l9[7:8, :], T, BF16)
        _flatten(nc, dram, bh, l4[2:3, :], T, BF16)
        _flatten(nc, dram, bm, l4[3:4, :], T, BF16)

    # ---- row max on every-4th row (bf16 K=4) ----
    r9 = per_s.tile([9, D], BF16, tag="r9")
    nc.vector.tensor_copy(out=r9[0:8, :], in_=r9c[0:8, :])
    if "max" not in SKIP:
        mq = per_s.tile([128, QT], F32, tag="mq")
        for q in range(QT):
            pm = pbig.tile([128, D], F32, tag="pbig")
            for c in range(2):
                nc.tensor.matmul(pm[:, 512 * c:512 * (c + 1)],
                                 a4d[:, 128 * q:128 * (q + 1)],
                                 l4[:, 512 * c:512 * (c + 1)],
                                 start=True, stop=True)
            nc.vector.tensor_reduce(out=mq[:, q:q + 1], in_=pm[:, :],
                                    axis=mybir.AxisListType.X, op=ALU.max)
        mqb = per_s.tile([128, QT], BF16, tag="mqb")
        nc.vector.tensor_scalar_mul(out=mqb, in0=mq, scalar1=-1.0)
        mrow = per_s.tile([1, 128 * QT], BF16, tag="mrow")
        _flatten(nc, dram, mqb, mrow, QT, BF16)
        mquad = per_s.tile([1, D], BF16, tag="mquad")
        mapr = mrow[0:1, :]
        nc.vector.tensor_copy(
            out=mquad.rearrange("r (k four) -> r k four", four=MAXSTRIDE),
            in_=bass.AP(tensor=mapr.tensor, offset=mapr.offset,
                        ap=[mapr.ap[0], [1, 128 * QT], [0, MAXSTRIDE]]))
        nc.sync.dma_start(out=r9[8:9, :], in_=mquad)
    return l9, r9, sw3


def _sample_back(nc, tc, b, front, per_s, epool, dram, pbig, pnd,
                 nhT, nlT, denT):
    l9, r9, sw3 = front
    # ---- argexp (K=9 bf16) + exp + num/den ----
    nd = pnd.tile([3, D], F32, tag="pnd")
    for g in range(T):
        if "argexp" not in SKIP:
            pa = pbig.tile([128, D], F32, tag="pbig")
            for c in range(2):
                nc.tensor.matmul(pa[:, 512 * c:512 * (c + 1)],
                                 l9[:, 128 * g:128 * (g + 1)],
                                 r9[:, 512 * c:512 * (c + 1)],
                                 start=True, stop=True)
        et = epool.tile([128, D], BF16, tag="et")
        if "exp" not in SKIP and "argexp" not in SKIP:
            nc.scalar.activation(out=et, in_=pa, func=ACTF.Exp)
        else:
            nc.vector.tensor_copy(out=et[:, 0:4], in_=sw3[:, 0:4])
        if "numden" not in SKIP:
            for c in range(2):
                nc.tensor.matmul(nd[:, 512 * c:512 * (c + 1)],
                                 sw3[:, 3 * g:3 * g + 3],
                                 et[:, 512 * c:512 * (c + 1)],
                                 start=(g == 0), stop=(g == T - 1))

    # rows (num_h, num_l, den) -> SBUF -> DRAM -> columns (col g*S + b)
    ndsb = per_s.tile([3, D], F32, tag="ndsb")
    nc.vector.tensor_copy(out=ndsb, in_=nd)
    for r, dst in ((0, nhT), (1, nlT), (2, denT)):
        scr = dram.tile([1, D], F32, tag=f"ndscr{r}")
        nc.scalar.dma_start(out=scr, in_=ndsb[r:r + 1, :])
        sap = scr[0:1, :]
        nc.scalar.dma_start(
            out=dst[:, b::S],
            in_=bass.AP(tensor=sap.tensor, offset=sap.offset,
                        ap=[[1, 128], [128, T]]))


def _mlp(nc, tc, per_s, big, dram, pmlp, nhT, nlT, denT,
         wtiles, w3sb, b1r, b2r, b3r, onesb, ones8, id8, out_t):
    # xs = (num_h + num_l) / den, in column form [128, S*T]
    rden = big.tile([128, S * T], F32, tag="rden")
    nc.vector.reciprocal(out=rden, in_=denT)
    nsum = big.tile([128, S * T], F32, tag="nsum")
    nc.vector.tensor_add(out=nsum, in0=nhT, in1=nlT)
    xsT = big.tile([128, S * T], F32, tag="xsT")
    nc.vector.tensor_mul(out=xsT, in0=rden, in1=nsum)

    hT = xsT
    for li, (wh, wl, brr) in enumerate((("w1h", "w1l", b1r), ("w2h", "w2l", b2r))):
        wh, wl = wtiles[wh], wtiles[wl]
        # split activations into bf16 parts
        hTh = big.tile([128, S * T], BF16, tag="hTh")
        nc.vector.tensor_copy(out=hTh, in_=hT)
        hTlf = big.tile([128, S * T], F32, tag="hTlf")
        nc.vector.tensor_sub(out=hTlf, in0=hT, in1=hTh)
        hTl = big.tile([128, S * T], BF16, tag="hTl")
        nc.vector.tensor_copy(out=hTl, in_=hTlf)

        hp = pmlp.tile([S, D], F32, tag="hp")
        for c in range(2):
            first = True
            for g in range(T):
                for lt, wt in ((hTh, wh), (hTh, wl), (hTl, wh)):
                    nc.tensor.matmul(hp[:, 512 * c:512 * (c + 1)],
                                     lt[:, g * S:(g + 1) * S],
                                     wt[:, g * D + 512 * c:g * D + 512 * (c + 1)],
                                     start=first, stop=False)
                    first = False
            nc.tensor.matmul(hp[:, 512 * c:512 * (c + 1)], onesb,
                             brr[:, 512 * c:512 * (c + 1)],
                             start=False, stop=True)
        # h rows -> SBUF fp32 -> column form via PE transpose
        hs = big.tile([S, D], F32, tag="hs")
        nc.vector.tensor_copy(out=hs, in_=hp)
        hTn = big.tile([128, S * T], F32, tag="hTn")
        for g in range(T):
            pt = pmlp.tile([128, S], F32, tag="pt")
            nc.tensor.transpose(pt, hs[:, 128 * g:128 * (g + 1)], id8)
            nc.vector.tensor_copy(out=hTn[:, g * S:(g + 1) * S], in_=pt)
        # leaky in column form: h = 0.01*h + relu(0.99*h)
        r99 = big.tile([128, S * T], F32, tag="r99")
        nc.scalar.activation(out=r99, in_=hTn, func=ACTF.Relu,
                             scale=1.0 - NEG_SLOPE)
        hTf = big.tile([128, S * T], F32, tag="hTf")
        nc.vector.scalar_tensor_tensor(out=hTf, in0=hTn, scalar=NEG_SLOPE,
                                       in1=r99, op0=ALU.mult, op1=ALU.add)
        hT = hTf

    op = pmlp.tile([S, 2], F32, tag="op")
    for g in range(T):
        nc.tensor.matmul(op, hT[:, g * S:(g + 1) * S], w3sb[:, 2 * g:2 * g + 2],
                         start=(g == 0), stop=False)
    nc.tensor.matmul(op, ones8, b3r[:, :], start=False, stop=True)
    osb = big.tile([S, 2], F32, tag="osb")
    nc.vector.tensor_copy(out=osb, in_=op)
    nc.sync.dma_start(out=out_t[:, :], in_=osb)


# ---------------------------------------------------------------------------
# host-side input prep + entry point
# ---------------------------------------------------------------------------

def make_in_maps(x, W1, b1, W2, b2, W3, b3):
    import ml_dtypes
    BF = ml_dtypes.bfloat16
    x = np.ascontiguousarray(x, dtype=np.float32)
    scaling = (D - 1 - 2 * np.arange(D)).astype(np.float32)
    a_h, a_l = bf_split(scaling, 2)
    neg1 = -np.ones(D, BF)
    r9c = np.stack([a_h, a_h, a_h, a_l, a_l, neg1, neg1, neg1,
                    np.zeros(D, BF)]).astype(BF)
    a_ev = np.ascontiguousarray(scaling[::MAXSTRIDE]).astype(BF)
    a4d = np.stack([a_ev, a_ev, -np.ones(D // MAXSTRIDE, BF),
                    -np.ones(D // MAXSTRIDE, BF)]).astype(BF)
    onesb = np.ones((1, S), BF)
    ones8 = np.ones((1, S), np.float32)
    w1h, w1l = bf_split(np.ascontiguousarray(W1.T, np.float32), 2)
    w2h, w2l = bf_split(np.ascontiguousarray(W2.T, np.float32), 2)
    w3t = np.ascontiguousarray(W3.T, dtype=np.float32)
    b1r = np.asarray(b1, np.float32).reshape(1, D).astype(BF)
    b2r = np.asarray(b2, np.float32).reshape(1, D).astype(BF)
    b3r = np.ascontiguousarray(np.asarray(b3, np.float32).reshape(1, 2))

    in_maps = []
    for c in range(NCORES):
        xs = x[c * S:(c + 1) * S]                      # [S, D]
        t = xs / TAU
        t_h, t_m, t_l = bf_split(t, 3)
        l9 = np.zeros((S, 9, D), BF)
        l9[:, 0], l9[:, 1], l9[:, 2] = t_h, t_m, t_l
        l9[:, 3], l9[:, 4] = t_h, t_m
        l9[:, 8] = 1.0
        l4 = np.zeros((S, 4, D), BF)
        l4[:, 0], l4[:, 1] = t_h, t_m
        cols = xs.reshape(S, T, 128).transpose(0, 2, 1)  # [S, 128, T]
        swg = np.ascontiguousarray(cols / TAU).astype(np.float32)
        s_h, s_l = bf_split(cols, 2)
        sw3 = np.zeros((S, 128, 3 * T), BF)
        sw3[:, :, 0::3] = s_h
        sw3[:, :, 1::3] = s_l
        sw3[:, :, 2::3] = 1.0
        in_maps.append({
            "xs8": np.ascontiguousarray(xs / TAU),
            "l9i": l9, "l4i": l4,
            "swg": swg, "sw3": sw3,
            "r9c": r9c, "a4d": a4d, "onesb": onesb, "ones8": ones8,
            "w1h": w1h, "w1l": w1l, "w2h": w2h, "w2l": w2l,
            "w3t": w3t, "b1r": b1r, "b2r": b2r, "b3r": b3r,
            "id8": np.eye(S, dtype=np.float32),
        })
    return in_maps


_NC_CACHE = {}


def get_nc(loop_n: int = 1):
    if loop_n not in _NC_CACHE:
        _NC_CACHE[loop_n] = build_nc(loop_n)
    return _NC_CACHE[loop_n]


def kernel(x, W1, b1, W2, b2, W3, b3):
    nc = get_nc()
    in_maps = make_in_maps(np.asarray(x), np.asarray(W1), np.asarray(b1),
                           np.asarray(W2), np.asarray(b2), np.asarray(W3),
                           np.asarray(b3))
    res = run_bass_kernel_spmd(nc, in_maps, core_ids=list(range(NCORES)))
    return np.concatenate([res.results[c]["out"] for c in range(NCORES)], axis=0)
